# revision 1
# baseline (speedup 1.0000x reference)
"""BitLinear (ternary-weight + 8-bit-activation quantized matmul) on 8 TRN2 cores.

Strategy: data-parallel over tokens. Each core gets 2048 of the 16384 tokens
plus the full weight matrix, computes the whole BitLinear forward for its
token shard on device, and the host concatenates the shards.

Math (must match the jax reference):
  w_scale = max(mean(|W|), 1e-6)                       (scalar)
  w_q     = clip(round(W / w_scale), -1, 1)            (ternary)
  a       = clip(max_i |x|, 1e-8, inf)                 (per token)
  x_q     = clip(round(x * 127 / a), -127, 127)        (8-bit ints)
  y       = (x_q @ w_q^T) * w_scale * a / 127

All rounding is done with the fp32 magic-number trick (v + 1.5*2^23 - 1.5*2^23
is round-to-nearest-even), so device results bit-match jnp.round. x_q (ints
<= 127) and w_q ({-1,0,1}) are exact in bf16 and products accumulate exactly
in fp32 PSUM, so the bf16 TensorE matmul is exact.

w_scale is extremely sensitive: a 2e-4 relative deviation flips enough
ternary weights near the .5 boundaries to push the max-normalized error to
3e-2 (measured), so no sampled/bf16 shortcut is possible — pass 1 must
abs-sum the full fp32 W.  Cross-core AllReduce of sharded partial sums costs
~80us on this fabric (measured), so each core does the full pass itself.

Final schedule (356-378us measured vs the 434us baseline; run-to-run
variance +-20us from shared-HBM contention).  The kernel is DMA-bound, so
the schedule is built around cutting DMA bytes and overlapping the forced
serial prefix:
  - the host feeds W TRANSPOSED (wT[k, out]).  mean|W| doesn't care about
    layout, and quantized wT tiles are directly the matmul moving operand
    => the 8.4MB wq on-chip DMA transpose disappears entirely.
  - quantized wq tiles are stored as fp8e4 (ternary {-1,0,1} is exact in
    e4m3; mixed bf16-stationary x fp8-moving matmul runs at bf16 speed),
    halving the resident wq pool to 32KB/partition.  That buys residency
    for the LEFT HALF (out cols 0:1024) of every fp32 wT tile; only the
    right halves are re-read after pass 1 (single-consumer tiles => no
    pool-recycling deadlock).  W traffic: 16.8 + 8.4 = 25.2MB.
  - quantization runs on [128,1024] column halves; the left halves are
    ready right after w_scale and the GEMM ramp (column-major over the
    first 4 token tiles) chases the quantize pipeline per-half (matmul b
    waits only the half containing its columns), so the PE starts ~87us
    in.  After the ramp the steady state runs gap-free at ~225ns/matmul.
  - pass-1 abs-sums alternate Act (flow tiles, in-place Abs + accum_out)
    and DVE (resident strips, reduce_sum) so neither engine throttles the
    stream; 4 flow bufs ride out the DMA->abs->DMA semaphore latency.
  - y is stored bf16 (host upcasts): rel err 3.0e-3 total, halves store
    traffic.
Dead ends (measured): sampled/bf16 w_scale (flips, 1.7-4e-2 err), sharded
pass-1 + AllReduce (~80us collective), fp8 DoubleRow (needs 2x physical
FLOPs for exact hi/lo split at only ~1.44x rate), per-quarter y stores
(HWDGE fixed overhead), longer 5-tile ramp, DVE-path quantize strips.
"""

from contextlib import ExitStack

import numpy as np

import concourse.bass as bass
import concourse.tile as tile
from concourse import bacc, bass_isa, mybir
from concourse.bass import ds, ts
from concourse.bass_utils import run_bass_kernel_spmd

F32 = mybir.dt.float32
BF16 = mybir.dt.bfloat16
FP8 = mybir.dt.float8e4
AF = mybir.ActivationFunctionType
OP = mybir.AluOpType
AX = mybir.AxisListType

B, S, D_IN, D_OUT = 4, 4096, 2048, 2048
N_CORES = 8
TOK = B * S                # 16384 tokens
TPC = TOK // N_CORES       # 2048 tokens per core
NT = TPC // 128            # 16 token tiles per core
NJ = D_IN // 128           # 16 wT k-tiles
NI = D_IN // 128           # 16 contraction (k) blocks
NO = D_OUT // 512          # 4 output column blocks
HALF = D_OUT // 2          # 1024: resident left half of wT columns
CM = 12582912.0            # 1.5 * 2^23: fp32 RNE rounding magic
QMAX = 127.0

KNOBS = {
    "y_bf16": True,
    "ldx_bufs": 2,
    "t1_bufs": 1,
    "xqt_bufs": 5,
    "ys_bufs": 4,          # full-row [128,2048] bf16 staging
    "wro_bufs": 4,         # rotating pass-1 flow tiles (cols 512:2048)
    "wrq_bufs": 3,         # rotating re-read strips
}

_CACHE = {}


def _emit(tc: tile.TileContext, x_d: bass.AP, w_d: bass.AP, y_d: bass.AP):
    nc = tc.nc
    ydt = BF16 if KNOBS["y_bf16"] else F32
    with ExitStack() as ctx:
        wres0 = ctx.enter_context(tc.tile_pool(name="wres0", bufs=1))
        wro = ctx.enter_context(tc.tile_pool(name="wro", bufs=KNOBS["wro_bufs"]))
        wrq = ctx.enter_context(tc.tile_pool(name="wrq", bufs=KNOBS["wrq_bufs"]))
        wqres = ctx.enter_context(tc.tile_pool(name="wqres", bufs=1))
        ldx = ctx.enter_context(tc.tile_pool(name="ldx", bufs=KNOBS["ldx_bufs"]))
        t1p = ctx.enter_context(tc.tile_pool(name="t1p", bufs=KNOBS["t1_bufs"]))
        t1w = ctx.enter_context(tc.tile_pool(name="t1w", bufs=3))
        xqp = ctx.enter_context(tc.tile_pool(name="xqp", bufs=2))
        xqtp = ctx.enter_context(tc.tile_pool(name="xqtp", bufs=KNOBS["xqt_bufs"]))
        ysp = ctx.enter_context(tc.tile_pool(name="ysp", bufs=KNOBS["ys_bufs"]))
        stats = ctx.enter_context(tc.tile_pool(name="stats", bufs=5))
        consts = ctx.enter_context(tc.tile_pool(name="consts", bufs=1))
        psum = ctx.enter_context(
            tc.tile_pool(name="psum", bufs=8, space=bass.MemorySpace.PSUM)
        )

        cpos = consts.tile([128, 1], F32, tag="cpos")
        nc.vector.memset(cpos, CM)
        czero = consts.tile([128, 1], F32, tag="czero")
        nc.vector.memset(czero, 0.0)
        # dummy activation on a ready constant: triggers the one-time
        # ACT_TABLE_LOAD during DMA warmup instead of on the critical chain
        warm = stats.tile([128, 1], F32, tag="warm")
        nc.scalar.activation(warm, czero, AF.Abs, bias=czero)

        xtiles = {}

        def x_load(t):
            xt = ldx.tile([128, D_IN], F32, tag="ldx", name=f"x{t}")
            nc.sync.dma_start(xt, x_d[ts(t, 128), :])
            xtiles[t] = xt

        xscales = {}

        def x_stats(t):
            xt = xtiles[t]
            a = stats.tile([128, 1], F32, tag="xa", name=f"xa{t}")
            nc.vector.reduce_max(a, xt, axis=AX.X, apply_absolute_value=True)
            nc.vector.tensor_scalar(a, a, 1e-8, None, OP.max)
            r0 = stats.tile([128, 1], F32, tag="xr0", name=f"xr0{t}")
            nc.vector.reciprocal(r0, a)
            ntt = stats.tile([128, 1], F32, tag="xntt", name=f"xntt{t}")
            nc.vector.tensor_mul(ntt, a, r0)
            nc.vector.tensor_scalar(ntt, ntt, -1.0, 2.0, OP.mult, OP.add)
            s = stats.tile([128, 1], F32, tag="xs", name=f"xs{t}")
            nc.vector.tensor_mul(s, r0, ntt)
            nc.vector.tensor_scalar(s, s, QMAX, None, OP.mult)  # 127/a
            xscales[t] = (a, s)

        xqts = {}

        def x_quant(t):
            a, s = xscales[t]
            t1 = t1p.tile([128, D_IN], F32, tag="t1", name=f"xt1_{t}")
            nc.scalar.activation(t1, xtiles.pop(t), AF.Identity, bias=cpos, scale=s)
            xq = xqp.tile([128, D_IN], BF16, tag="xq", name=f"xq{t}")
            nc.vector.tensor_scalar(xq, t1, -CM, None, OP.add)
            xqT = xqtp.tile([128, NI, 128], BF16, tag="xqT", name=f"xqT{t}")
            nc.scalar.dma_start(xqT, xq, transpose=True)
            xqts[t] = xqT

        souts = {}

        def x_sout(t, ws127):
            a, _ = xscales[t]
            sout = stats.tile([128, 1], F32, tag="xsout", name=f"xsout{t}")
            nc.scalar.activation(sout, a, AF.Identity, bias=czero, scale=ws127)
            souts[t] = sout

        # ---- pass 1: stream wT in column halves; left halves stay
        # resident, right halves flow through a small pool.  Each half is
        # abs-summed on arrival, alternating Act/DVE so neither engine
        # throttles the DMA stream.
        wsumsA = stats.tile([128, NJ], F32, tag="wsumsA")
        wsumsB = stats.tile([128, NJ], F32, tag="wsumsB")
        wres = {}

        def pass1(j):
            # resident left half (quarters 0-1): DVE reduce (doesn't clobber)
            # for the last tile, load the flow half first so the final
            # abs-sum on the w_scale critical chain is the cheaper DVE one
            def left():
                lt = wres0.tile([128, HALF], F32, tag=f"wl{j}", name=f"wl{j}")
                nc.sync.dma_start(lt, w_d[ts(j, 128), 0:HALF])
                wres[j] = lt
                nc.vector.reduce_sum(
                    wsumsB[:, ds(j, 1)], lt, axis=AX.X, apply_absolute_value=True
                )

            def right():
                rt = wro.tile([128, D_OUT - HALF], F32, tag="wro", name=f"wRp{j}")
                nc.sync.dma_start(rt, w_d[ts(j, 128), HALF:D_OUT])
                nc.scalar.activation(
                    rt, rt, AF.Abs, bias=czero, accum_out=wsumsA[:, ds(j, 1)]
                )

            if j == NJ - 1:
                right()
                left()
            else:
                left()
                right()

        x_load(0)
        x_load(1)
        pass1(0)
        pass1(1)
        x_stats(0)
        x_quant(0)
        pass1(2)
        pass1(3)
        x_stats(1)
        x_quant(1)
        for j in range(4, NJ):
            pass1(j)
        x_load(2)
        x_stats(2)
        x_quant(2)
        x_load(3)

        # ---- re-read streams for quarters 2-3, strip per (tile, quarter),
        # each consumed exactly once by its quantize ----
        wR = {}

        def r_load(j, no):
            rt = wrq.tile([128, 512], F32, tag="wrq", name=f"wR{j}_{no}")
            nc.sync.dma_start(rt, w_d[ts(j, 128), ds(no * 512, 512)])
            wR[(j, no)] = rt

        for j in range(NJ):
            r_load(j, 2)
        for j in range(NJ):
            r_load(j, 3)

        # ---- w_scale ----
        wsA = stats.tile([128, 1], F32, tag="wsA")
        nc.vector.reduce_sum(wsA, wsumsA, axis=AX.X)
        wsB = stats.tile([128, 1], F32, tag="wsB")
        nc.vector.reduce_sum(wsB, wsumsB, axis=AX.X)
        wsum_p = stats.tile([128, 1], F32, tag="wsp")
        nc.vector.tensor_add(wsum_p, wsA, wsB)
        wsum_all = stats.tile([128, 1], F32, tag="wsa")
        nc.gpsimd.partition_all_reduce(wsum_all, wsum_p, 128, bass_isa.ReduceOp.add)
        wscale = consts.tile([128, 1], F32, tag="wscale")
        nc.vector.tensor_scalar(
            wscale, wsum_all, 1.0 / (D_OUT * D_IN), 1e-6, OP.mult, OP.max
        )
        r0 = stats.tile([128, 1], F32, tag="wr0")
        nc.vector.reciprocal(r0, wscale)
        ntt = stats.tile([128, 1], F32, tag="wntt")
        nc.vector.tensor_mul(ntt, wscale, r0)
        nc.vector.tensor_scalar(ntt, ntt, -1.0, 2.0, OP.mult, OP.add)
        rws = consts.tile([128, 1], F32, tag="rws")
        nc.vector.tensor_mul(rws, r0, ntt)
        ws127 = consts.tile([128, 1], F32, tag="ws127")
        nc.vector.tensor_scalar(ws127, wscale, 1.0 / QMAX, None, OP.mult)

        # ---- W quantize: per-quarter column strips into resident wq ----
        # wq[b] is [128 k, 2048 out] bf16; strip (b, no) covers columns
        # [512*no, 512*no+512).  Quarters 0-1 come from resident left
        # halves, 2-3 from the re-read right halves.
        wq = [
            wqres.tile([128, D_OUT], FP8, tag=f"wq{b}", name=f"wq{b}")
            for b in range(NJ)
        ]

        def w_quant_half(b):
            t1 = t1w.tile([128, HALF], F32, tag="t1w", name=f"wtl_{b}")
            nc.scalar.activation(t1, wres[b], AF.Identity, bias=cpos, scale=rws)
            nc.vector.tensor_scalar(t1, t1, CM - 1.0, CM + 1.0, OP.max, OP.min)
            nc.vector.tensor_scalar(wq[b][:, 0:HALF], t1, -CM, None, OP.add)

        def w_quant_strip(b, no):
            t1 = t1w.tile([128, HALF], F32, tag="t1w", name=f"wtr_{b}_{no}")
            t1s = t1[:, 0:512]
            nc.scalar.activation(t1s, wR.pop((b, no)), AF.Identity, bias=cpos, scale=rws)
            nc.vector.tensor_scalar(t1s, t1s, CM - 1.0, CM + 1.0, OP.max, OP.min)
            nc.vector.tensor_scalar(
                wq[b][:, ds(no * 512, 512)], t1s, -CM, None, OP.add
            )

        # ---- GEMM ----
        ys = {}

        def cell(no, t):
            if t not in ys:
                ys[t] = ysp.tile([128, D_OUT], ydt, tag="ys", name=f"ys{t}")
            ps = psum.tile([128, 512], F32, tag="ps")
            xqT = xqts[t]
            for b in range(NI):
                nc.tensor.matmul(
                    ps,
                    xqT[:, b, :],
                    wq[b][:, ds(no * 512, 512)],
                    start=(b == 0),
                    stop=(b == NI - 1),
                )
            nc.vector.tensor_scalar(ys[t][:, ts(no, 512)], ps, souts[t], None, OP.mult)

        def y_store(t):
            nc.sync.dma_start(y_d[ts(t, 128), :], ys.pop(t))
            del xqts[t]

        # ramp: quantize quarter no, then run it over the first 4 token
        # tiles while the next quarter quantizes.
        RAMP = 4
        for b in range(NJ):
            w_quant_half(b)
        x_stats(3)
        x_quant(3)
        x_load(4)
        for t in range(RAMP):
            x_sout(t, ws127)
        for no in range(2):
            for t in range(RAMP):
                cell(no, t)
        for no in range(2, NO):
            for b in range(NJ):
                w_quant_strip(b, no)
            for t in range(RAMP):
                cell(no, t)
        for t in range(RAMP):
            y_store(t)

        # steady state: everything resident
        for t in range(4, NT):
            if t + 1 < NT:
                x_load(t + 1)
            x_stats(t)
            x_quant(t)
            x_sout(t, ws127)
            for no in range(NO):
                cell(no, t)
            y_store(t)


def _build():
    key = tuple(sorted((k, str(v)) for k, v in KNOBS.items()))
    if key in _CACHE:
        return _CACHE[key]
    nc = bacc.Bacc(
        "TRN2", target_bir_lowering=False, debug=False, num_devices=N_CORES
    )
    x_d = nc.dram_tensor("x", [TPC, D_IN], F32, kind="ExternalInput").ap()
    # w is fed TRANSPOSED by the host: [k, out]
    w_d = nc.dram_tensor("w", [D_IN, D_OUT], F32, kind="ExternalInput").ap()
    ydt = BF16 if KNOBS["y_bf16"] else F32
    y_d = nc.dram_tensor("y", [TPC, D_OUT], ydt, kind="ExternalOutput").ap()
    with tile.TileContext(nc) as tc:
        _emit(tc, x_d, w_d, y_d)
    nc.compile()
    _CACHE[key] = nc
    return nc


_last_result = None  # BassKernelResults of the most recent run (for profiling)


def kernel(x: np.ndarray, weight: np.ndarray, trace: bool = False) -> np.ndarray:
    global _last_result
    nc = _build()
    xf = np.ascontiguousarray(x.reshape(TOK, D_IN), dtype=np.float32)
    wT = np.ascontiguousarray(weight.T, dtype=np.float32)
    in_maps = [
        {"x": xf[c * TPC:(c + 1) * TPC], "w": wT}
        for c in range(N_CORES)
    ]
    res = run_bass_kernel_spmd(nc, in_maps, list(range(N_CORES)), trace=trace)
    _last_result = res
    y = np.concatenate(
        [np.asarray(res.results[c]["y"]) for c in range(N_CORES)], axis=0
    )
    return y.reshape(B, S, D_OUT).astype(np.float32)



# revision 2
# speedup vs baseline: 1.0579x; 1.0579x over previous
"""BitLinear (ternary-weight + 8-bit-activation quantized matmul) on 8 TRN2 cores.

Strategy: data-parallel over tokens. Each core gets 2048 of the 16384 tokens
plus the full weight matrix, computes the whole BitLinear forward for its
token shard on device, and the host concatenates the shards.

Math (must match the jax reference):
  w_scale = max(mean(|W|), 1e-6)                       (scalar)
  w_q     = clip(round(W / w_scale), -1, 1)            (ternary)
  a       = clip(max_i |x|, 1e-8, inf)                 (per token)
  x_q     = clip(round(x * 127 / a), -127, 127)        (8-bit ints)
  y       = (x_q @ w_q^T) * w_scale * a / 127

v2 design (vs the 360-394us v1; roofline: PE 221us GEMM + ~52us W stream):
  - w_scale is extremely sensitive (2e-4 rel deviation flips ternary weights
    near .5 boundaries -> 3e-2 err), so pass 1 must abs-sum the full fp32 W.
    W is fed TRANSPOSED (wT[k,out]) and is FULLY RESIDENT in fp32 (128KB of
    the ~208KB/partition SBUF): no re-read at all, W streams exactly once
    (16 x 1MB DMAs all issued upfront with zero pool waits -> line rate).
  - the x side runs in BF16: x is cast f32->bf16 during the DMA itself
    (SWDGE/gpsimd ring, separate from the W stream's HWDGE ring). a_scale
    and x_q both derive from the same bf16 x, so the a in the quantize and
    the a in the rescale cancel; measured 7.0e-3 total err (budget 2e-2).
    That halves x SBUF and doubles DVE reduce throughput.
  - rounding uses the fp32 magic-number trick (+1.5*2^23) on ACT (bias=CM,
    scale per-partition), then one DVE op subtracts CM. x_q needs no clamp
    (|x*127/a| <= 127 by construction). w_q subtracts into fp8 directly
    (ints <= ~8 are exact in e4m3) and clamps IN-PLACE on the fp8 tile
    (8-bit DVE op, cheap). wq tiles are fp8e4 (ternary exact; bf16 x fp8
    matmul runs at bf16 speed): 32KB resident.
  - GEMM ramp: right after w_scale, 8 PSUM cells (tiles 0-1 x 4 col-blocks)
    accumulate b-blocks in lockstep with the wq quantize stream (PE chases
    at 1.7us/b vs ~2.2us/b produce rate), so the PE starts ~2us after
    w_scale instead of waiting for all of wq.
  - steady state: software-pipelined one tile ahead (stats/quant/transpose
    of tile t+1 run during the cells of tile t; x loads issue two ahead).
    Per-iter budget: PE 13.8us, DVE ~5us, ACT ~2.3us, sync 2 DMA issues.
  - y is stored bf16 (host upcasts): halves store traffic.
Dead ends (measured): sampled/bf16 w_scale (1.7-4e-2 err), sharded pass-1 +
AllReduce (~80us collective), fp8 DoubleRow x_q (needs exact hi/lo split =
2x FLOPs at only ~1.44x rate; single-pass fp8 x_q approx = 2.3e-2 err >
budget), per-quarter y stores (HWDGE fixed overhead).
"""

from contextlib import ExitStack

import numpy as np

import concourse.bass as bass
import concourse.tile as tile
from concourse import bacc, bass_isa, mybir
from concourse.bass import ds, ts
from concourse.bass_utils import run_bass_kernel_spmd

F32 = mybir.dt.float32
BF16 = mybir.dt.bfloat16
FP8 = mybir.dt.float8e4
AF = mybir.ActivationFunctionType
OP = mybir.AluOpType
AX = mybir.AxisListType

B, S, D_IN, D_OUT = 4, 4096, 2048, 2048
N_CORES = 8
TOK = B * S                # 16384 tokens
TPC = TOK // N_CORES       # 2048 tokens per core
NT = TPC // 128            # 16 token tiles per core
NB = D_IN // 128           # 16 contraction (k) blocks
NO = D_OUT // 512          # 4 output column blocks
CM = 12582912.0            # 1.5 * 2^23: fp32 RNE rounding magic
QMAX = 127.0

KNOBS = {
    "ldx_bufs": 2,
    "xq_bufs": 1,
    "t1_bufs": 2,
    "xqt_bufs": 2,
    "ys_bufs": 2,
    "psum_bufs": 8,
}

_CACHE = {}


def _emit(tc: tile.TileContext, x_d: bass.AP, w_d: bass.AP, y_d: bass.AP):
    nc = tc.nc
    with ExitStack() as ctx:
        wres = ctx.enter_context(tc.tile_pool(name="wres", bufs=1))
        wqp = ctx.enter_context(tc.tile_pool(name="wqp", bufs=1))
        ldx = ctx.enter_context(tc.tile_pool(name="ldx", bufs=KNOBS["ldx_bufs"]))
        xqp = ctx.enter_context(tc.tile_pool(name="xqp", bufs=KNOBS["xq_bufs"]))
        xqtp = ctx.enter_context(tc.tile_pool(name="xqtp", bufs=KNOBS["xqt_bufs"]))
        ysp = ctx.enter_context(tc.tile_pool(name="ysp", bufs=KNOBS["ys_bufs"]))
        t1p = ctx.enter_context(tc.tile_pool(name="t1p", bufs=KNOBS["t1_bufs"]))
        stats = ctx.enter_context(tc.tile_pool(name="stats", bufs=4))
        consts = ctx.enter_context(tc.tile_pool(name="consts", bufs=1))
        psum = ctx.enter_context(
            tc.tile_pool(name="psum", bufs=KNOBS["psum_bufs"], space=bass.MemorySpace.PSUM)
        )

        cpos = consts.tile([128, 1], F32, tag="cpos")
        nc.vector.memset(cpos, CM)
        czero = consts.tile([128, 1], F32, tag="czero")
        nc.vector.memset(czero, 0.0)
        # dummy activation on a ready constant: triggers the one-time
        # ACT_TABLE_LOAD during DMA warmup instead of on the critical chain
        warm = stats.tile([128, 1], F32, tag="warm")
        nc.scalar.activation(warm, czero, AF.Abs, bias=czero)

        # ---- issue the ENTIRE W stream upfront: 16 x 1MB, all resident,
        # no pool recycling -> the sync ring drains at HBM line rate.
        wt = []
        for j in range(NB):
            t = wres.tile([128, D_OUT], F32, tag=f"w{j}", name=f"w{j}")
            nc.sync.dma_start(t, w_d[ts(j, 128), :])
            wt.append(t)

        xtiles = {}

        def x_load(t):
            xt = ldx.tile([128, D_IN], BF16, tag="ldx", name=f"x{t}")
            nc.gpsimd.dma_start(xt, x_d[ts(t, 128), :])  # f32 -> bf16 cast DMA
            xtiles[t] = xt

        x_load(0)
        x_load(1)

        # pass-1 abs-sums on DVE (reduce doesn't clobber the resident W)
        wsums = stats.tile([128, NB], F32, tag="wsums")

        def pass1(j):
            nc.vector.reduce_sum(
                wsums[:, ds(j, 1)], wt[j], axis=AX.X, apply_absolute_value=True
            )

        xscales = {}

        def x_stats(t):
            a = stats.tile([128, 1], F32, tag="xa", name=f"xa{t}")
            nc.vector.reduce_max(a, xtiles[t], axis=AX.X, apply_absolute_value=True)
            nc.vector.tensor_scalar(a, a, 1e-8, None, OP.max)
            r0 = stats.tile([128, 1], F32, tag="xr0", name=f"xr0{t}")
            nc.vector.reciprocal(r0, a)
            ntt = stats.tile([128, 1], F32, tag="xntt", name=f"xntt{t}")
            nc.vector.tensor_mul(ntt, a, r0)
            nc.vector.tensor_scalar(ntt, ntt, -1.0, 2.0, OP.mult, OP.add)
            s = stats.tile([128, 1], F32, tag="xs", name=f"xs{t}")
            nc.vector.tensor_mul(s, r0, ntt)
            nc.vector.tensor_scalar(s, s, QMAX, None, OP.mult)  # 127/a
            xscales[t] = (a, s)

        xqts = {}

        def x_quant(t):
            a, s = xscales[t]
            t1 = t1p.tile([128, D_IN], F32, tag="t1", name=f"xt1_{t}")
            nc.scalar.activation(t1, xtiles.pop(t), AF.Identity, bias=cpos, scale=s)
            xq = xqp.tile([128, D_IN], BF16, tag="xq", name=f"xq{t}")
            nc.vector.tensor_scalar(xq, t1, -CM, None, OP.add)
            xqT = xqtp.tile([128, NB, 128], BF16, tag="xqT", name=f"xqT{t}")
            nc.sync.dma_start(xqT, xq, transpose=True)
            xqts[t] = xqT

        souts = {}

        def x_sout(t):
            a, _ = xscales[t]
            so = stats.tile([128, 1], F32, tag="xso", name=f"xso{t}")
            nc.vector.tensor_scalar(so, a, ws127, None, OP.mult)
            souts[t] = so

        # interleave pass-1 with x prep for the first two tiles
        pass1(0)
        pass1(1)
        pass1(2)
        x_stats(0)
        x_quant(0)
        pass1(3)
        pass1(4)
        pass1(5)
        x_stats(1)
        x_quant(1)
        for j in range(6, NB):
            pass1(j)
        x_load(2)
        x_load(3)
        # stats for tiles 2,3 run cheap on DVE now; their quant (ACT-side)
        # is deferred past the wq quantize stream
        x_stats(2)
        x_stats(3)

        # ---- w_scale ----
        wsA = stats.tile([128, 1], F32, tag="wsA")
        nc.vector.reduce_sum(wsA, wsums, axis=AX.X)
        wsum_all = stats.tile([128, 1], F32, tag="wsa")
        nc.gpsimd.partition_all_reduce(wsum_all, wsA, 128, bass_isa.ReduceOp.add)
        wscale = consts.tile([128, 1], F32, tag="wscale")
        nc.vector.tensor_scalar(
            wscale, wsum_all, 1.0 / (D_OUT * D_IN), 1e-6, OP.mult, OP.max
        )
        r0 = stats.tile([128, 1], F32, tag="wr0")
        nc.vector.reciprocal(r0, wscale)
        ntt = stats.tile([128, 1], F32, tag="wntt")
        nc.vector.tensor_mul(ntt, wscale, r0)
        nc.vector.tensor_scalar(ntt, ntt, -1.0, 2.0, OP.mult, OP.add)
        rws = consts.tile([128, 1], F32, tag="rws")
        nc.vector.tensor_mul(rws, r0, ntt)
        ws127 = consts.tile([128, 1], F32, tag="ws127")
        nc.vector.tensor_scalar(ws127, wscale, 1.0 / QMAX, None, OP.mult)
        x_sout(0)
        x_sout(1)

        # ---- W quantize stream + PE chase-ramp ----
        # wq[b] = clip(round(wT[b]/ws), -1, 1) as fp8: ACT magic-round ->
        # DVE subtract CM into fp8 (ints <= 8 exact) -> DVE in-place clamp.
        wq = [
            wqp.tile([128, D_OUT], FP8, tag=f"wq{b}", name=f"wq{b}")
            for b in range(NB)
        ]

        def w_quant(b):
            t1 = t1p.tile([128, D_OUT], F32, tag="t1", name=f"wt1_{b}")
            nc.scalar.activation(t1, wt[b], AF.Identity, bias=cpos, scale=rws)
            nc.vector.tensor_scalar(wq[b], t1, -CM, None, OP.add)
            nc.vector.tensor_scalar(wq[b], wq[b], -1.0, 1.0, OP.max, OP.min)

        for b in range(NB):
            w_quant(b)

        # 8 PSUM cells (tiles 0-1 x col-blocks 0-3) accumulate each b as its
        # wq lands; PE consumes at ~1.7us/b vs ~2.2us/b quantize rate.
        chase = [(t, no) for t in range(2) for no in range(NO)]
        pss = {}
        for c, (t, no) in enumerate(chase):
            pss[c] = psum.tile([128, 512], F32, tag="ps", name=f"cps{c}")
        for b in range(NB):
            for c, (t, no) in enumerate(chase):
                nc.tensor.matmul(
                    pss[c],
                    xqts[t][:, b, :],
                    wq[b][:, ds(no * 512, 512)],
                    start=(b == 0),
                    stop=(b == NB - 1),
                )

        # x prep for tiles 2,3 (ACT is free again after the wq stream)
        x_quant(2)
        x_quant(3)
        x_sout(2)
        x_sout(3)
        x_load(4)

        ys = {}

        def y_tile(t):
            if t not in ys:
                ys[t] = ysp.tile([128, D_OUT], BF16, tag="ys", name=f"ys{t}")
            return ys[t]

        for c, (t, no) in enumerate(chase):
            nc.vector.tensor_scalar(
                y_tile(t)[:, ds(no * 512, 512)], pss[c], souts[t], None, OP.mult
            )
        del pss

        def y_store(t):
            nc.sync.dma_start(y_d[ts(t, 128), :], ys.pop(t))
            del xqts[t]

        y_store(0)
        y_store(1)

        # ---- steady state: one tile of x-prep lookahead ----
        def cell(no, t):
            ps = psum.tile([128, 512], F32, tag="ps")
            xqT = xqts[t]
            for b in range(NB):
                nc.tensor.matmul(
                    ps,
                    xqT[:, b, :],
                    wq[b][:, ds(no * 512, 512)],
                    start=(b == 0),
                    stop=(b == NB - 1),
                )
            nc.vector.tensor_scalar(
                y_tile(t)[:, ds(no * 512, 512)], ps, souts[t], None, OP.mult
            )

        for t in range(2, NT):
            if t + 3 < NT:
                x_load(t + 3)
            if t + 2 < NT:
                x_stats(t + 2)
                x_quant(t + 2)
                x_sout(t + 2)
            for no in range(NO):
                cell(no, t)
            y_store(t)


def _build():
    key = tuple(sorted((k, str(v)) for k, v in KNOBS.items()))
    if key in _CACHE:
        return _CACHE[key]
    nc = bacc.Bacc(
        "TRN2", target_bir_lowering=False, debug=False, num_devices=N_CORES
    )
    x_d = nc.dram_tensor("x", [TPC, D_IN], F32, kind="ExternalInput").ap()
    # w is fed TRANSPOSED by the host: [k, out]
    w_d = nc.dram_tensor("w", [D_IN, D_OUT], F32, kind="ExternalInput").ap()
    y_d = nc.dram_tensor("y", [TPC, D_OUT], BF16, kind="ExternalOutput").ap()
    with tile.TileContext(nc) as tc:
        _emit(tc, x_d, w_d, y_d)
    nc.compile()
    _CACHE[key] = nc
    return nc


_last_result = None  # BassKernelResults of the most recent run (for profiling)


def kernel(x: np.ndarray, weight: np.ndarray, trace: bool = False) -> np.ndarray:
    global _last_result
    nc = _build()
    xf = np.ascontiguousarray(x.reshape(TOK, D_IN), dtype=np.float32)
    wT = np.ascontiguousarray(weight.T, dtype=np.float32)
    in_maps = [
        {"x": xf[c * TPC:(c + 1) * TPC], "w": wT}
        for c in range(N_CORES)
    ]
    res = run_bass_kernel_spmd(nc, in_maps, list(range(N_CORES)), trace=trace)
    _last_result = res
    y = np.concatenate(
        [np.asarray(res.results[c]["y"]) for c in range(N_CORES)], axis=0
    )
    return y.reshape(B, S, D_OUT).astype(np.float32)
